# revision 39
# baseline (speedup 1.0000x reference)
"""TRN2 Bass kernel for nn_BetweenClusterFC.

Computes out[n] = sum_f (emb_1 @ W1 + b1)[n,f] * (emb_2 @ W2 + b2)[n,f]
for emb_1/emb_2 [32768, 1024] fp32, W [1024, 512], b [512], out [32768] fp32.

Sharding: data-parallel over the 8 NeuronCores — each core handles 4096 rows;
W1/W2 replicated. No cross-core communication; outputs concatenated on the
host.

Strategy (single-pass fp16, DMA-descriptor-lean):
  - The rel-err budget for this problem is 2e-2; single-pass fp16 matmuls
    land at 3.4e-4 (measured on HW), so one fp16 matmul per (tile, kc,
    input) = 512 MMs of N=512 per core runs at the warm-PE stream roofline
    (216 ns/MM measured; 110.6us of stream per core).
  - Embeddings are relaid out host-side to [group, p, kc, r] so each e-DMA
    is 128 descriptors of >=2KB contiguous per partition (a plain [D, N]
    transpose gives 1024x512B descriptors, which cost 1.6us of sync-engine
    issue time each and stalled the PE). W1/W2 are interleaved host-side to
    [p, kc, j, f] and DMA'd in per-kc chunks in consumption order: startup
    is HBM-paced (~3MB must land before tile 0 completes, ~8.5us after the
    ~7us NEFF preamble), and the SDMA round-robins queued transfers, so
    fine chunks in consumption order minimize the PE wait.
  - Per 128-row tile: 16 j-interleaved matmuls accumulate h1/h2 into two
    PSUM banks (bufs=3 rotation so the PE never waits on the reducers);
    the scalar engine stages h1 PSUM->SBUF (the DVE may read only one PSUM
    operand), then DVE tensor_tensor + tensor_reduce write sum_f(h1*h2)
    into acc[:, rt]. (A fused scalar_tensor_tensor/tensor_tensor_reduce
    crashed the exec unit on HW, so the two-op form stays.) Biases are
    zero in this problem; a general bias variant compiles only if b != 0.
  - acc [128 rows-in-tile, 32 tiles] is DMA'd out raw and transposed on
    the host (free), eliminating the PE-transpose + copy + strided-store
    tail.
  - fp16 warmup matmuls on a zeroed tile (alternating two PSUM banks)
    bridge the startup-DMA window so real matmuls start at the
    un-throttled 2.4 GHz PE clock (HAM re-throttles after ~3.4us idle).
  - Measured: 133.7us HW exec (vs 368.6us baseline). Note the chip
    sometimes sits in a ~2.0 GHz power state (P0), where the same kernel
    measures ~158us; both states are ~stream-roofline-bound.
"""

import sys
import time

import numpy as np

if "/opt/trn_rl_repo" not in sys.path:
    sys.path.insert(0, "/opt/trn_rl_repo")

import concourse.mybir as mybir
import concourse.tile as tile
from concourse import bacc
from concourse.bass_utils import run_bass_kernel_spmd

F32 = mybir.dt.float32
F16 = mybir.dt.float16

N = 32768
D = 1024
F = 512
P = 128
NCORES = 8
R = N // NCORES   # rows per core
RT = R // P       # 128-row tiles per core
KC = D // P       # contraction chunks
NFIRST = 2        # single-tile e-DMA groups at the head (fast first arrival)
GRP = 2           # row-tiles per e-DMA group for the rest
NG = (RT - NFIRST) // GRP  # rest e-DMA groups
NWARM = 8         # fp16 warmup matmuls bridging the startup DMA window
DEPTH = 3         # A/B software-pipeline depth (== h_psum bufs; 6 banks)
FUSE_STT = False  # fused DVE mult+reduce crashed HW intermittently; keep off

_CACHE = {}


def _build_program(with_bias=False, rows=R):
    rt_count = rows // P
    ng = (rt_count - NFIRST) // GRP
    nc = bacc.Bacc("TRN2", target_bir_lowering=False, debug=False)

    def din(name, shape, dt=F16):
        return nc.dram_tensor(name, shape, dt, kind="ExternalInput").ap()

    # host-prearranged layouts: e [group, p, kc, r-in-group], w [p, kc, f]
    e1f = din("e1f", [NFIRST, P, KC, P])
    e2f = din("e2f", [NFIRST, P, KC, P])
    e1h = din("e1h", [ng, P, KC, GRP * P])
    e2h = din("e2h", [ng, P, KC, GRP * P])
    # W1/W2 interleaved per-kc so one DMA delivers both inputs' chunk k
    wb = din("wb", [P, KC, 2, F])
    if with_bias:
        b1 = din("b1", [F], F32)
        b2 = din("b2", [F], F32)
    # out[p, rt] = result for row rt*128+p; transposed host-side. One extra
    # column holds the last tile's second half-reduction (host adds it).
    out = nc.dram_tensor(
        "out", [P, rt_count + 1], F32, kind="ExternalOutput").ap()

    mult = mybir.AluOpType.mult
    add = mybir.AluOpType.add

    with tile.TileContext(nc) as tc:
        with (
            tc.tile_pool(name="consts", bufs=1) as consts,
            tc.tile_pool(name="epool", bufs=1) as epool,
            tc.tile_pool(name="hpool", bufs=2) as hpool,
            tc.tile_pool(name="fin", bufs=1) as fin_pool,
            tc.tile_pool(name="w_psum", bufs=1, space="PSUM") as w_psum,
            tc.tile_pool(name="h_psum", bufs=3, space="PSUM") as h_psum,
        ):
            wsb = consts.tile([P, KC, 2, F], F16, tag="wsb")
            ef = [
                [epool.tile([P, KC, P], F16, tag=f"e{j}f{t}", name=f"e{j}f{t}")
                 for t in range(NFIRST)]
                for j in range(2)
            ]
            eg = [
                [epool.tile([P, KC, GRP * P], F16, tag=f"e{j}g{g}",
                            name=f"e{j}g{g}")
                 for g in range(ng)]
                for j in range(2)
            ]

            # Startup is HBM-paced (~3MB before any tile can fully finish,
            # ~8.5us after the ~7us preamble) and the SDMA round-robins all
            # queued transfers, so delivery order ~= issue order at ~1us per
            # 256KB chunk. Issue in consumption order of the software-
            # pipelined A/B schedule below: first e tiles + kc0-3 weight
            # chunks, then the rest. Each DMA is 128 descriptors of >=2KB
            # contiguous per partition.
            # Weight chunks alternate between the two HWDGE rings (sync +
            # scalar) so their per-DMA overheads overlap and the e tiles
            # don't stall the weight drip: tile 0 consumes a chunk-pair
            # every 0.43us, which a single ring (~0.85us/chunk) can't feed.
            # wc0 leads the sync ring so the first real matmul is gated by
            # the e tiles (~11us), not a weight chunk queued behind them.
            nc.sync.dma_start(wsb[:, 0], wb[:, 0])
            nc.scalar.dma_start(wsb[:, 1], wb[:, 1])
            nc.sync.dma_start(ef[0][0][:], e1f[0])
            nc.scalar.dma_start(wsb[:, 3], wb[:, 3])
            nc.sync.dma_start(ef[1][0][:], e2f[0])
            nc.scalar.dma_start(wsb[:, 5], wb[:, 5])
            nc.sync.dma_start(wsb[:, 2], wb[:, 2])
            nc.scalar.dma_start(wsb[:, 7], wb[:, 7])
            nc.sync.dma_start(wsb[:, 4], wb[:, 4])
            nc.sync.dma_start(wsb[:, 6], wb[:, 6])
            for t in range(1, NFIRST):
                nc.sync.dma_start(ef[0][t][:], e1f[t])
                nc.sync.dma_start(ef[1][t][:], e2f[t])
            for g in range(ng):
                nc.sync.dma_start(eg[0][g][:], e1h[g])
                nc.sync.dma_start(eg[1][g][:], e2h[g])

            if with_bias:
                b1_bc = consts.tile([P, F], F32, tag="b1")
                nc.gpsimd.dma_start(b1_bc[:], b1[None, :].to_broadcast((P, F)))
                b2_bc = consts.tile([P, F], F32, tag="b2")
                nc.gpsimd.dma_start(b2_bc[:], b2[None, :].to_broadcast((P, F)))

            # fp16 warmup matmuls bridge the startup-DMA window so the first
            # real matmuls run at the un-throttled PE clock; alternate two
            # PSUM banks so consecutive warmups overlap fill/drain
            warm16 = consts.tile([P, F], F16, tag="warm16")
            nc.vector.memset(warm16[:], 0.0)
            warm_ps = [w_psum.tile([P, F], F32, tag=f"warm{i}", name=f"warm{i}")
                       for i in range(2)]
            for i in range(NWARM):
                nc.tensor.matmul(
                    warm_ps[i % 2][:], lhsT=warm16[:, :P], rhs=warm16[:],
                    start=True, stop=True,
                )

            acc = fin_pool.tile([P, rt_count + 1], F32, tag="acc")
            nc.gpsimd.memset(acc[:, rt_count:], 0.0)

            def lhs_of(rt, j, kc):
                if rt < NFIRST:
                    return ef[j][rt][:, kc, :]
                g, ri = divmod(rt - NFIRST, GRP)
                return eg[j][g][:, kc, ri * P:(ri + 1) * P]

            def mm_half(rt, hps, klo, khi, jmajor=False):
                # jmajor: finish j=0's accumulation first so the scalar-
                # engine h0 copy (and then the DVE chain) starts ~0.9us
                # before the half ends — the pipelined A-half that reuses
                # these PSUM banks then never waits on the DVE.
                order = ([(j, kc) for j in range(2)
                          for kc in range(klo, khi)] if jmajor else
                         [(j, kc) for kc in range(klo, khi)
                          for j in range(2)])
                for j, kc in order:
                    nc.tensor.matmul(
                        hps[j][:],
                        lhsT=lhs_of(rt, j, kc),
                        rhs=wsb[:, kc, j, :],
                        start=(kc == 0),
                        stop=(kc == KC - 1),
                    )

            def finish_tile_split(rt, hps):
                # last tile: pipeline the reduce in F-halves so the output
                # DMA issues ~0.7us sooner. The two half-reductions land in
                # acc[:, rt] and acc[:, rt+1]; the host adds them.
                for lo, hi, col in ((0, F // 2, rt), (F // 2, F, rt + 1)):
                    h0sb = hpool.tile([P, F // 2], F32, tag=f"h0s{lo}",
                                      name=f"h0s{lo}_{rt}")
                    nc.scalar.activation(
                        h0sb[:], hps[0][:, lo:hi],
                        mybir.ActivationFunctionType.Copy)
                    prod = hpool.tile([P, F // 2], F32, tag=f"pr{lo}",
                                      name=f"pr{lo}_{rt}")
                    nc.vector.tensor_tensor(
                        prod[:], h0sb[:], hps[1][:, lo:hi], mult)
                    nc.vector.tensor_reduce(
                        acc[:, col:col + 1], prod[:],
                        axis=mybir.AxisListType.X, op=add,
                    )

            def finish_tile(rt, hps):
                if with_bias:
                    hts = []
                    for j, b_bc in enumerate((b1_bc, b2_bc)):
                        ht = hpool.tile([P, F], F32, tag=f"ht{j}",
                                        name=f"ht{j}_{rt}")
                        nc.vector.tensor_tensor(ht[:], hps[j][:], b_bc[:], add)
                        hts.append(ht)
                    in0, in1 = hts[0][:], hts[1][:]
                else:
                    # DVE can read at most one PSUM operand; stage h0 in SBUF
                    # via the scalar engine (close to PSUM, off the DVE path)
                    h0sb = hpool.tile([P, F], F32, tag="h0sb",
                                      name=f"h0sb_{rt}")
                    nc.scalar.activation(
                        h0sb[:], hps[0][:], mybir.ActivationFunctionType.Copy)
                    in0, in1 = h0sb[:], hps[1][:]

                prod = hpool.tile([P, F], F32, tag="prod", name=f"prod_{rt}")
                if FUSE_STT:
                    nc.vector.scalar_tensor_tensor(
                        prod[:], in0, 1.0, in1, op0=mult, op1=mult,
                        accum_out=acc[:, rt:rt + 1],
                    )
                else:
                    nc.vector.tensor_tensor(prod[:], in0, in1, mult)
                    nc.vector.tensor_reduce(
                        acc[:, rt:rt + 1], prod[:],
                        axis=mybir.AxisListType.X, op=add,
                    )

            # Flat tile loop (measured faster than an A/B K-half software
            # pipeline, which traded startup idle for steady-state PSUM
            # reuse stalls): per tile, 16 j-interleaved matmuls accumulate
            # h1/h2 into two PSUM banks, then the scalar+vector engines
            # reduce into acc while the next tile's matmuls run.
            for rt in range(rt_count):
                hps = [
                    h_psum.tile([P, F], F32, tag=f"h{j}", name=f"hp{j}_{rt}")
                    for j in range(2)
                ]
                mm_half(rt, hps, 0, KC)
                if rt == rt_count - 1 and not with_bias:
                    finish_tile_split(rt, hps)
                else:
                    finish_tile(rt, hps)

            nc.sync.dma_start(out, acc[:])

    nc.compile()
    return nc


def _get_program(with_bias=False):
    key = ("bias" if with_bias else "fast")
    if key not in _CACHE:
        _CACHE[key] = _build_program(with_bias=with_bias)
    return _CACHE[key]


def _prep_e(emb):
    # [N, D] fp32 -> per-core ([NFIRST, p, kc, 128], [ng, p, kc, GRP*128])
    # fp16; contiguous per (group, partition) for 128-descriptor DMAs
    et = np.ascontiguousarray(
        np.asarray(emb, dtype=np.float32).T).astype(np.float16)
    # et [D, N]: [kc*128+p, c*R + rt*P + r]
    v = et.reshape(KC, P, NCORES, RT, P).transpose(2, 3, 1, 0, 4)
    # v [c, rt, p, kc, r]
    first = np.ascontiguousarray(v[:, :NFIRST])
    rest = np.ascontiguousarray(
        v[:, NFIRST:].reshape(NCORES, NG, GRP, P, KC, P)
        .transpose(0, 1, 3, 4, 2, 5)
        .reshape(NCORES, NG, P, KC, GRP * P))
    return first, rest


def _prep_w(W1, W2):
    # -> [p, kc, j, f] fp16: one 2KB-contiguous chunk per (partition, kc)
    w = np.stack([
        np.asarray(W1, dtype=np.float32).astype(np.float16),
        np.asarray(W2, dtype=np.float32).astype(np.float16),
    ])  # [j, kc*128+p, f]
    return np.ascontiguousarray(
        w.reshape(2, KC, P, F).transpose(2, 1, 0, 3))


def make_in_maps(emb_1, emb_2, W1, b1, W2, b2, with_bias=False):
    e1f, e1r = _prep_e(emb_1)
    e2f, e2r = _prep_e(emb_2)
    wb = _prep_w(W1, W2)
    maps = []
    for c in range(NCORES):
        m = {"e1f": e1f[c], "e2f": e2f[c], "e1h": e1r[c], "e2h": e2r[c],
             "wb": wb}
        if with_bias:
            m["b1"] = np.ascontiguousarray(np.asarray(b1, dtype=np.float32))
            m["b2"] = np.ascontiguousarray(np.asarray(b2, dtype=np.float32))
        maps.append(m)
    return maps


def kernel(emb_1, emb_2, W1, b1, W2, b2, **_unused):
    with_bias = bool(np.any(np.asarray(b1)) or np.any(np.asarray(b2)))
    nc = _get_program(with_bias)
    in_maps = make_in_maps(emb_1, emb_2, W1, b1, W2, b2, with_bias=with_bias)
    last_err = None
    for attempt in range(3):
        try:
            res = run_bass_kernel_spmd(nc, in_maps, list(range(NCORES))).results
            # out[p, rt] -> rows rt*128+p; fold in the last tile's second
            # half-reduction from the extra column
            outs = []
            for c in range(NCORES):
                raw = np.asarray(res[c]["out"])
                core = raw[:, :RT].copy()
                core[:, RT - 1] += raw[:, RT]
                outs.append(core.T.reshape(R))
            return np.concatenate(outs)
        except Exception as e:  # transient NRT/axon failures observed; retry
            last_err = e
            time.sleep(2.0 * (attempt + 1))
    raise last_err


# revision 40
# speedup vs baseline: 1.0012x; 1.0012x over previous
"""TRN2 Bass kernel for nn_BetweenClusterFC.

Computes out[n] = sum_f (emb_1 @ W1 + b1)[n,f] * (emb_2 @ W2 + b2)[n,f]
for emb_1/emb_2 [32768, 1024] fp32, W [1024, 512], b [512], out [32768] fp32.

Sharding: data-parallel over the 8 NeuronCores — each core handles 4096 rows;
W1/W2 replicated. No cross-core communication; outputs concatenated on the
host.

Strategy (single-pass fp16, DMA-descriptor-lean):
  - The rel-err budget for this problem is 2e-2; single-pass fp16 matmuls
    land at 3.4e-4 (measured on HW), so one fp16 matmul per (tile, kc,
    input) = 512 MMs of N=512 per core runs at the warm-PE stream roofline
    (216 ns/MM measured; 110.6us of stream per core).
  - Embeddings are relaid out host-side to [group, p, kc, r] so each e-DMA
    is 128 descriptors of >=2KB contiguous per partition (a plain [D, N]
    transpose gives 1024x512B descriptors, which cost 1.6us of sync-engine
    issue time each and stalled the PE). W1/W2 are interleaved host-side to
    [p, kc, j, f] and DMA'd in per-kc chunks in consumption order: startup
    is HBM-paced (~3MB must land before tile 0 completes, ~8.5us after the
    ~7us NEFF preamble), and the SDMA round-robins queued transfers, so
    fine chunks in consumption order minimize the PE wait.
  - Per 128-row tile: 16 j-interleaved matmuls accumulate h1/h2 into two
    PSUM banks (bufs=3 rotation so the PE never waits on the reducers);
    the scalar engine stages h1 PSUM->SBUF (the DVE may read only one PSUM
    operand), then DVE tensor_tensor + tensor_reduce write sum_f(h1*h2)
    into acc[:, rt]. (A fused scalar_tensor_tensor/tensor_tensor_reduce
    crashed the exec unit on HW, so the two-op form stays.) Biases are
    zero in this problem; a general bias variant compiles only if b != 0.
  - acc [128 rows-in-tile, 32 tiles] is DMA'd out raw and transposed on
    the host (free), eliminating the PE-transpose + copy + strided-store
    tail.
  - fp16 warmup matmuls on a zeroed tile (alternating two PSUM banks)
    bridge the startup-DMA window so real matmuls start at the
    un-throttled 2.4 GHz PE clock (HAM re-throttles after ~3.4us idle).
  - Measured: 133.7us HW exec (vs 368.6us baseline). Note the chip
    sometimes sits in a ~2.0 GHz power state (P0), where the same kernel
    measures ~158us; both states are ~stream-roofline-bound.
"""

import sys
import time

import numpy as np

if "/opt/trn_rl_repo" not in sys.path:
    sys.path.insert(0, "/opt/trn_rl_repo")

import concourse.mybir as mybir
import concourse.tile as tile
from concourse import bacc
from concourse.bass_utils import run_bass_kernel_spmd

F32 = mybir.dt.float32
F16 = mybir.dt.float16

N = 32768
D = 1024
F = 512
P = 128
NCORES = 8
R = N // NCORES   # rows per core
RT = R // P       # 128-row tiles per core
KC = D // P       # contraction chunks
NFIRST = 2        # single-tile e-DMA groups at the head (fast first arrival)
GRP = 2           # row-tiles per e-DMA group for the rest
NG = (RT - NFIRST) // GRP  # rest e-DMA groups
NWARM = 8         # fp16 warmup matmuls bridging the startup DMA window
DEPTH = 3         # A/B software-pipeline depth (== h_psum bufs; 6 banks)
FUSE_STT = False  # fused DVE mult+reduce crashed HW intermittently; keep off

_CACHE = {}


def _build_program(with_bias=False, rows=R):
    rt_count = rows // P
    ng = (rt_count - NFIRST) // GRP
    nc = bacc.Bacc("TRN2", target_bir_lowering=False, debug=False)

    def din(name, shape, dt=F16):
        return nc.dram_tensor(name, shape, dt, kind="ExternalInput").ap()

    # host-prearranged layouts: e [group, p, kc, r-in-group], w [p, kc, f]
    e1f = din("e1f", [NFIRST, P, KC, P])
    e2f = din("e2f", [NFIRST, P, KC, P])
    e1h = din("e1h", [ng, P, KC, GRP * P])
    e2h = din("e2h", [ng, P, KC, GRP * P])
    # W1/W2 interleaved per-kc so one DMA delivers both inputs' chunk k
    wb = din("wb", [P, KC, 2, F])
    if with_bias:
        b1 = din("b1", [F], F32)
        b2 = din("b2", [F], F32)
    # out[p, rt] = result for row rt*128+p; transposed host-side. One extra
    # column holds the last tile's second half-reduction (host adds it).
    out = nc.dram_tensor(
        "out", [P, rt_count + 1], F32, kind="ExternalOutput").ap()

    mult = mybir.AluOpType.mult
    add = mybir.AluOpType.add

    with tile.TileContext(nc) as tc:
        with (
            tc.tile_pool(name="consts", bufs=1) as consts,
            tc.tile_pool(name="epool", bufs=1) as epool,
            tc.tile_pool(name="hpool", bufs=2) as hpool,
            tc.tile_pool(name="fin", bufs=1) as fin_pool,
            tc.tile_pool(name="w_psum", bufs=1, space="PSUM") as w_psum,
            tc.tile_pool(name="h_psum", bufs=3, space="PSUM") as h_psum,
        ):
            wsb = consts.tile([P, KC, 2, F], F16, tag="wsb")
            ef = [
                [epool.tile([P, KC, P], F16, tag=f"e{j}f{t}", name=f"e{j}f{t}")
                 for t in range(NFIRST)]
                for j in range(2)
            ]
            eg = [
                [epool.tile([P, KC, GRP * P], F16, tag=f"e{j}g{g}",
                            name=f"e{j}g{g}")
                 for g in range(ng)]
                for j in range(2)
            ]

            # Startup is HBM-paced (~3MB before any tile can fully finish,
            # ~8.5us after the ~7us preamble) and the SDMA round-robins all
            # queued transfers, so delivery order ~= issue order at ~1us per
            # 256KB chunk. Issue in consumption order of the software-
            # pipelined A/B schedule below: first e tiles + kc0-3 weight
            # chunks, then the rest. Each DMA is 128 descriptors of >=2KB
            # contiguous per partition.
            # wc0/wc1 lead so the first real matmul is gated by the e tiles
            # (~11.5us), not a weight chunk queued behind them; e1f1/e2f1
            # slot in mid-weights so tile 1 never waits. (Splitting the
            # weight chunks across both HWDGE rings measured worse — the
            # rings share SDMA bandwidth, so it only reshuffles lateness.)
            nc.sync.dma_start(wsb[:, 0], wb[:, 0])
            nc.sync.dma_start(wsb[:, 1], wb[:, 1])
            nc.sync.dma_start(ef[0][0][:], e1f[0])
            nc.sync.dma_start(ef[1][0][:], e2f[0])
            nc.sync.dma_start(wsb[:, 2], wb[:, 2])
            nc.sync.dma_start(wsb[:, 3], wb[:, 3])
            for t in range(1, NFIRST):
                nc.sync.dma_start(ef[0][t][:], e1f[t])
                nc.sync.dma_start(ef[1][t][:], e2f[t])
            for k in range(4, KC):
                nc.sync.dma_start(wsb[:, k], wb[:, k])
            for g in range(ng):
                nc.sync.dma_start(eg[0][g][:], e1h[g])
                nc.sync.dma_start(eg[1][g][:], e2h[g])

            if with_bias:
                b1_bc = consts.tile([P, F], F32, tag="b1")
                nc.gpsimd.dma_start(b1_bc[:], b1[None, :].to_broadcast((P, F)))
                b2_bc = consts.tile([P, F], F32, tag="b2")
                nc.gpsimd.dma_start(b2_bc[:], b2[None, :].to_broadcast((P, F)))

            # fp16 warmup matmuls bridge the startup-DMA window so the first
            # real matmuls run at the un-throttled PE clock; alternate two
            # PSUM banks so consecutive warmups overlap fill/drain
            warm16 = consts.tile([P, F], F16, tag="warm16")
            nc.vector.memset(warm16[:], 0.0)
            warm_ps = [w_psum.tile([P, F], F32, tag=f"warm{i}", name=f"warm{i}")
                       for i in range(2)]
            for i in range(NWARM):
                nc.tensor.matmul(
                    warm_ps[i % 2][:], lhsT=warm16[:, :P], rhs=warm16[:],
                    start=True, stop=True,
                )

            acc = fin_pool.tile([P, rt_count + 1], F32, tag="acc")
            nc.gpsimd.memset(acc[:, rt_count:], 0.0)

            def lhs_of(rt, j, kc):
                if rt < NFIRST:
                    return ef[j][rt][:, kc, :]
                g, ri = divmod(rt - NFIRST, GRP)
                return eg[j][g][:, kc, ri * P:(ri + 1) * P]

            def mm_half(rt, hps, klo, khi, jmajor=False):
                # jmajor: finish j=0's accumulation first so the scalar-
                # engine h0 copy (and then the DVE chain) starts ~0.9us
                # before the half ends — the pipelined A-half that reuses
                # these PSUM banks then never waits on the DVE.
                order = ([(j, kc) for j in range(2)
                          for kc in range(klo, khi)] if jmajor else
                         [(j, kc) for kc in range(klo, khi)
                          for j in range(2)])
                for j, kc in order:
                    nc.tensor.matmul(
                        hps[j][:],
                        lhsT=lhs_of(rt, j, kc),
                        rhs=wsb[:, kc, j, :],
                        start=(kc == 0),
                        stop=(kc == KC - 1),
                    )

            def finish_tile_split(rt, hps):
                # last tile: pipeline the reduce in F-halves so the output
                # DMA issues ~0.7us sooner. The two half-reductions land in
                # acc[:, rt] and acc[:, rt+1]; the host adds them.
                for lo, hi, col in ((0, F // 2, rt), (F // 2, F, rt + 1)):
                    h0sb = hpool.tile([P, F // 2], F32, tag=f"h0s{lo}",
                                      name=f"h0s{lo}_{rt}")
                    nc.scalar.activation(
                        h0sb[:], hps[0][:, lo:hi],
                        mybir.ActivationFunctionType.Copy)
                    prod = hpool.tile([P, F // 2], F32, tag=f"pr{lo}",
                                      name=f"pr{lo}_{rt}")
                    nc.vector.tensor_tensor(
                        prod[:], h0sb[:], hps[1][:, lo:hi], mult)
                    nc.vector.tensor_reduce(
                        acc[:, col:col + 1], prod[:],
                        axis=mybir.AxisListType.X, op=add,
                    )

            def finish_tile(rt, hps):
                if with_bias:
                    hts = []
                    for j, b_bc in enumerate((b1_bc, b2_bc)):
                        ht = hpool.tile([P, F], F32, tag=f"ht{j}",
                                        name=f"ht{j}_{rt}")
                        nc.vector.tensor_tensor(ht[:], hps[j][:], b_bc[:], add)
                        hts.append(ht)
                    in0, in1 = hts[0][:], hts[1][:]
                else:
                    # DVE can read at most one PSUM operand; stage h0 in SBUF
                    # via the scalar engine (close to PSUM, off the DVE path)
                    h0sb = hpool.tile([P, F], F32, tag="h0sb",
                                      name=f"h0sb_{rt}")
                    nc.scalar.activation(
                        h0sb[:], hps[0][:], mybir.ActivationFunctionType.Copy)
                    in0, in1 = h0sb[:], hps[1][:]

                prod = hpool.tile([P, F], F32, tag="prod", name=f"prod_{rt}")
                if FUSE_STT:
                    nc.vector.scalar_tensor_tensor(
                        prod[:], in0, 1.0, in1, op0=mult, op1=mult,
                        accum_out=acc[:, rt:rt + 1],
                    )
                else:
                    nc.vector.tensor_tensor(prod[:], in0, in1, mult)
                    nc.vector.tensor_reduce(
                        acc[:, rt:rt + 1], prod[:],
                        axis=mybir.AxisListType.X, op=add,
                    )

            # Flat tile loop (measured faster than an A/B K-half software
            # pipeline, which traded startup idle for steady-state PSUM
            # reuse stalls): per tile, 16 j-interleaved matmuls accumulate
            # h1/h2 into two PSUM banks, then the scalar+vector engines
            # reduce into acc while the next tile's matmuls run.
            for rt in range(rt_count):
                hps = [
                    h_psum.tile([P, F], F32, tag=f"h{j}", name=f"hp{j}_{rt}")
                    for j in range(2)
                ]
                mm_half(rt, hps, 0, KC)
                if rt == rt_count - 1 and not with_bias:
                    finish_tile_split(rt, hps)
                else:
                    finish_tile(rt, hps)

            nc.sync.dma_start(out, acc[:])

    nc.compile()
    return nc


def _get_program(with_bias=False):
    key = ("bias" if with_bias else "fast")
    if key not in _CACHE:
        _CACHE[key] = _build_program(with_bias=with_bias)
    return _CACHE[key]


def _prep_e(emb):
    # [N, D] fp32 -> per-core ([NFIRST, p, kc, 128], [ng, p, kc, GRP*128])
    # fp16; contiguous per (group, partition) for 128-descriptor DMAs
    et = np.ascontiguousarray(
        np.asarray(emb, dtype=np.float32).T).astype(np.float16)
    # et [D, N]: [kc*128+p, c*R + rt*P + r]
    v = et.reshape(KC, P, NCORES, RT, P).transpose(2, 3, 1, 0, 4)
    # v [c, rt, p, kc, r]
    first = np.ascontiguousarray(v[:, :NFIRST])
    rest = np.ascontiguousarray(
        v[:, NFIRST:].reshape(NCORES, NG, GRP, P, KC, P)
        .transpose(0, 1, 3, 4, 2, 5)
        .reshape(NCORES, NG, P, KC, GRP * P))
    return first, rest


def _prep_w(W1, W2):
    # -> [p, kc, j, f] fp16: one 2KB-contiguous chunk per (partition, kc)
    w = np.stack([
        np.asarray(W1, dtype=np.float32).astype(np.float16),
        np.asarray(W2, dtype=np.float32).astype(np.float16),
    ])  # [j, kc*128+p, f]
    return np.ascontiguousarray(
        w.reshape(2, KC, P, F).transpose(2, 1, 0, 3))


def make_in_maps(emb_1, emb_2, W1, b1, W2, b2, with_bias=False):
    e1f, e1r = _prep_e(emb_1)
    e2f, e2r = _prep_e(emb_2)
    wb = _prep_w(W1, W2)
    maps = []
    for c in range(NCORES):
        m = {"e1f": e1f[c], "e2f": e2f[c], "e1h": e1r[c], "e2h": e2r[c],
             "wb": wb}
        if with_bias:
            m["b1"] = np.ascontiguousarray(np.asarray(b1, dtype=np.float32))
            m["b2"] = np.ascontiguousarray(np.asarray(b2, dtype=np.float32))
        maps.append(m)
    return maps


def kernel(emb_1, emb_2, W1, b1, W2, b2, **_unused):
    with_bias = bool(np.any(np.asarray(b1)) or np.any(np.asarray(b2)))
    nc = _get_program(with_bias)
    in_maps = make_in_maps(emb_1, emb_2, W1, b1, W2, b2, with_bias=with_bias)
    last_err = None
    for attempt in range(3):
        try:
            res = run_bass_kernel_spmd(nc, in_maps, list(range(NCORES))).results
            # out[p, rt] -> rows rt*128+p; fold in the last tile's second
            # half-reduction from the extra column
            outs = []
            for c in range(NCORES):
                raw = np.asarray(res[c]["out"])
                core = raw[:, :RT].copy()
                core[:, RT - 1] += raw[:, RT]
                outs.append(core.T.reshape(R))
            return np.concatenate(outs)
        except Exception as e:  # transient NRT/axon failures observed; retry
            last_err = e
            time.sleep(2.0 * (attempt + 1))
    raise last_err


# revision 41
# speedup vs baseline: 1.0132x; 1.0120x over previous
"""TRN2 Bass kernel for nn_BetweenClusterFC.

Computes out[n] = sum_f (emb_1 @ W1 + b1)[n,f] * (emb_2 @ W2 + b2)[n,f]
for emb_1/emb_2 [32768, 1024] fp32, W [1024, 512], b [512], out [32768] fp32.

Sharding: data-parallel over the 8 NeuronCores — each core handles 4096 rows;
W1/W2 replicated. No cross-core communication; outputs concatenated on the
host.

Strategy (single-pass fp16, DMA-descriptor-lean):
  - The rel-err budget for this problem is 2e-2; single-pass fp16 matmuls
    land at 3.4e-4 (measured on HW), so one fp16 matmul per (tile, kc,
    input) = 512 MMs of N=512 per core runs at the warm-PE stream roofline
    (216 ns/MM measured; 110.6us of stream per core).
  - Embeddings are relaid out host-side to [group, p, kc, r] so each e-DMA
    is 128 descriptors of >=2KB contiguous per partition (a plain [D, N]
    transpose gives 1024x512B descriptors, which cost 1.6us of sync-engine
    issue time each and stalled the PE). W1/W2 are interleaved host-side to
    [p, kc, j, f] and DMA'd in per-kc chunks in consumption order: startup
    is HBM-paced (~3MB must land before tile 0 completes, ~8.5us after the
    ~7us NEFF preamble), and the SDMA round-robins queued transfers, so
    fine chunks in consumption order minimize the PE wait.
  - Per 128-row tile: 16 j-interleaved matmuls accumulate h1/h2 into two
    PSUM banks (bufs=3 rotation so the PE never waits on the reducers);
    the scalar engine stages h1 PSUM->SBUF (the DVE may read only one PSUM
    operand), then DVE tensor_tensor + tensor_reduce write sum_f(h1*h2)
    into acc[:, rt]. (A fused scalar_tensor_tensor/tensor_tensor_reduce
    crashed the exec unit on HW, so the two-op form stays.) Biases are
    zero in this problem; a general bias variant compiles only if b != 0.
  - acc [128 rows-in-tile, 32 tiles] is DMA'd out raw and transposed on
    the host (free), eliminating the PE-transpose + copy + strided-store
    tail.
  - fp16 warmup matmuls on a zeroed tile (alternating two PSUM banks)
    bridge the startup-DMA window so real matmuls start at the
    un-throttled 2.4 GHz PE clock (HAM re-throttles after ~3.4us idle).
  - Measured: 133.7us HW exec (vs 368.6us baseline). Note the chip
    sometimes sits in a ~2.0 GHz power state (P0), where the same kernel
    measures ~158us; both states are ~stream-roofline-bound.
"""

import sys
import time

import numpy as np

if "/opt/trn_rl_repo" not in sys.path:
    sys.path.insert(0, "/opt/trn_rl_repo")

import concourse.mybir as mybir
import concourse.tile as tile
from concourse import bacc
from concourse.bass_utils import run_bass_kernel_spmd

F32 = mybir.dt.float32
F16 = mybir.dt.float16

N = 32768
D = 1024
F = 512
P = 128
NCORES = 8
R = N // NCORES   # rows per core
RT = R // P       # 128-row tiles per core
KC = D // P       # contraction chunks
NFIRST = 2        # single-tile e-DMA groups at the head (fast first arrival)
GRP = 2           # row-tiles per e-DMA group for the rest
NG = (RT - NFIRST) // GRP  # rest e-DMA groups
NWARM = 8         # fp16 warmup matmuls bridging the startup DMA window
DEPTH = 3         # A/B software-pipeline depth (== h_psum bufs; 6 banks)
FUSE_STT = False  # fused DVE mult+reduce crashed HW intermittently; keep off

_CACHE = {}


def _build_program(with_bias=False, rows=R):
    rt_count = rows // P
    ng = (rt_count - NFIRST) // GRP
    nc = bacc.Bacc("TRN2", target_bir_lowering=False, debug=False)

    def din(name, shape, dt=F16):
        return nc.dram_tensor(name, shape, dt, kind="ExternalInput").ap()

    # host-prearranged layouts: e [group, p, kc, r-in-group], w [p, kc, f]
    e1f = din("e1f", [NFIRST, P, KC, P])
    e2f = din("e2f", [NFIRST, P, KC, P])
    e1h = din("e1h", [ng, P, KC, GRP * P])
    e2h = din("e2h", [ng, P, KC, GRP * P])
    # W1/W2 interleaved per-kc so one DMA delivers both inputs' chunk k
    wb = din("wb", [P, KC, 2, F])
    if with_bias:
        b1 = din("b1", [F], F32)
        b2 = din("b2", [F], F32)
    # out[p, rt] = result for row rt*128+p; transposed host-side. One extra
    # column holds the last tile's second half-reduction (host adds it).
    out = nc.dram_tensor(
        "out", [P, rt_count + 1], F32, kind="ExternalOutput").ap()

    mult = mybir.AluOpType.mult
    add = mybir.AluOpType.add

    with tile.TileContext(nc) as tc:
        with (
            tc.tile_pool(name="consts", bufs=1) as consts,
            tc.tile_pool(name="epool", bufs=1) as epool,
            tc.tile_pool(name="hpool", bufs=2) as hpool,
            tc.tile_pool(name="fin", bufs=1) as fin_pool,
            tc.tile_pool(name="w_psum", bufs=1, space="PSUM") as w_psum,
            tc.tile_pool(name="h_psum", bufs=3, space="PSUM") as h_psum,
        ):
            wsb = consts.tile([P, KC, 2, F], F16, tag="wsb")
            ef = [
                [epool.tile([P, KC, P], F16, tag=f"e{j}f{t}", name=f"e{j}f{t}")
                 for t in range(NFIRST)]
                for j in range(2)
            ]
            eg = [
                [epool.tile([P, KC, GRP * P], F16, tag=f"e{j}g{g}",
                            name=f"e{j}g{g}")
                 for g in range(ng)]
                for j in range(2)
            ]

            # Startup is HBM-paced (~3MB before any tile can fully finish,
            # ~8.5us after the ~7us preamble) and the SDMA round-robins all
            # queued transfers, so delivery order ~= issue order at ~1us per
            # 256KB chunk. Issue in consumption order of the software-
            # pipelined A/B schedule below: first e tiles + kc0-3 weight
            # chunks, then the rest. Each DMA is 128 descriptors of >=2KB
            # contiguous per partition.
            # wc0/wc1 lead so the first real matmul is gated by the e tiles
            # (~11.5us), not a weight chunk queued behind them; e1f1/e2f1
            # slot in mid-weights so tile 1 never waits. (Splitting the
            # weight chunks across both HWDGE rings measured worse — the
            # rings share SDMA bandwidth, so it only reshuffles lateness.)
            nc.sync.dma_start(wsb[:, 0], wb[:, 0])
            nc.sync.dma_start(wsb[:, 1], wb[:, 1])
            nc.sync.dma_start(ef[0][0][:], e1f[0])
            nc.sync.dma_start(ef[1][0][:], e2f[0])
            nc.sync.dma_start(wsb[:, 2], wb[:, 2])
            nc.sync.dma_start(wsb[:, 3], wb[:, 3])
            for t in range(1, NFIRST):
                nc.sync.dma_start(ef[0][t][:], e1f[t])
                nc.sync.dma_start(ef[1][t][:], e2f[t])
            for k in range(4, KC):
                nc.sync.dma_start(wsb[:, k], wb[:, k])
            for g in range(ng):
                nc.sync.dma_start(eg[0][g][:], e1h[g])
                nc.sync.dma_start(eg[1][g][:], e2h[g])

            if with_bias:
                b1_bc = consts.tile([P, F], F32, tag="b1")
                nc.gpsimd.dma_start(b1_bc[:], b1[None, :].to_broadcast((P, F)))
                b2_bc = consts.tile([P, F], F32, tag="b2")
                nc.gpsimd.dma_start(b2_bc[:], b2[None, :].to_broadcast((P, F)))

            # fp16 warmup matmuls bridge the startup-DMA window so the first
            # real matmuls run at the un-throttled PE clock; alternate two
            # PSUM banks so consecutive warmups overlap fill/drain
            warm16 = consts.tile([P, F], F16, tag="warm16")
            nc.vector.memset(warm16[:], 0.0)
            warm_ps = [w_psum.tile([P, F], F32, tag=f"warm{i}", name=f"warm{i}")
                       for i in range(2)]
            for i in range(NWARM):
                nc.tensor.matmul(
                    warm_ps[i % 2][:], lhsT=warm16[:, :P], rhs=warm16[:],
                    start=True, stop=True,
                )

            acc = fin_pool.tile([P, rt_count + 1], F32, tag="acc")
            nc.gpsimd.memset(acc[:, rt_count:], 0.0)

            def lhs_of(rt, j, kc):
                if rt < NFIRST:
                    return ef[j][rt][:, kc, :]
                g, ri = divmod(rt - NFIRST, GRP)
                return eg[j][g][:, kc, ri * P:(ri + 1) * P]

            def mm_half(rt, hps, klo, khi, jmajor=False):
                # jmajor: finish j=0's accumulation first so the scalar-
                # engine h0 copy (and then the DVE chain) starts ~0.9us
                # before the half ends — the pipelined A-half that reuses
                # these PSUM banks then never waits on the DVE.
                order = ([(j, kc) for j in range(2)
                          for kc in range(klo, khi)] if jmajor else
                         [(j, kc) for kc in range(klo, khi)
                          for j in range(2)])
                for j, kc in order:
                    nc.tensor.matmul(
                        hps[j][:],
                        lhsT=lhs_of(rt, j, kc),
                        rhs=wsb[:, kc, j, :],
                        start=(kc == 0),
                        stop=(kc == KC - 1),
                    )

            def finish_tile_split(rt, hps):
                # last tile: pipeline the reduce in F-halves so the output
                # DMA issues ~0.7us sooner. The two half-reductions land in
                # acc[:, rt] and acc[:, rt+1]; the host adds them.
                for lo, hi, col in ((0, F // 2, rt), (F // 2, F, rt + 1)):
                    h0sb = hpool.tile([P, F // 2], F32, tag=f"h0s{lo}",
                                      name=f"h0s{lo}_{rt}")
                    nc.scalar.activation(
                        h0sb[:], hps[0][:, lo:hi],
                        mybir.ActivationFunctionType.Copy)
                    prod = hpool.tile([P, F // 2], F32, tag=f"pr{lo}",
                                      name=f"pr{lo}_{rt}")
                    nc.vector.tensor_tensor(
                        prod[:], h0sb[:], hps[1][:, lo:hi], mult)
                    nc.vector.tensor_reduce(
                        acc[:, col:col + 1], prod[:],
                        axis=mybir.AxisListType.X, op=add,
                    )

            def finish_tile(rt, hps):
                if with_bias:
                    hts = []
                    for j, b_bc in enumerate((b1_bc, b2_bc)):
                        ht = hpool.tile([P, F], F32, tag=f"ht{j}",
                                        name=f"ht{j}_{rt}")
                        nc.vector.tensor_tensor(ht[:], hps[j][:], b_bc[:], add)
                        hts.append(ht)
                    in0, in1 = hts[0][:], hts[1][:]
                else:
                    # DVE can read at most one PSUM operand; stage h0 in SBUF
                    # via the scalar engine (close to PSUM, off the DVE path)
                    h0sb = hpool.tile([P, F], F32, tag="h0sb",
                                      name=f"h0sb_{rt}")
                    nc.scalar.activation(
                        h0sb[:], hps[0][:], mybir.ActivationFunctionType.Copy)
                    in0, in1 = h0sb[:], hps[1][:]

                prod = hpool.tile([P, F], F32, tag="prod", name=f"prod_{rt}")
                if FUSE_STT:
                    nc.vector.scalar_tensor_tensor(
                        prod[:], in0, 1.0, in1, op0=mult, op1=mult,
                        accum_out=acc[:, rt:rt + 1],
                    )
                else:
                    nc.vector.tensor_tensor(prod[:], in0, in1, mult)
                    nc.vector.tensor_reduce(
                        acc[:, rt:rt + 1], prod[:],
                        axis=mybir.AxisListType.X, op=add,
                    )

            # Flat tile loop (measured faster than an A/B K-half software
            # pipeline, which traded startup idle for steady-state PSUM
            # reuse stalls): per tile, 16 j-interleaved matmuls accumulate
            # h1/h2 into two PSUM banks, then the scalar+vector engines
            # reduce into acc while the next tile's matmuls run.
            for rt in range(rt_count):
                hps = [
                    h_psum.tile([P, F], F32, tag=f"h{j}", name=f"hp{j}_{rt}")
                    for j in range(2)
                ]
                mm_half(rt, hps, 0, KC)
                if rt == rt_count - 1 and not with_bias:
                    finish_tile_split(rt, hps)
                else:
                    finish_tile(rt, hps)
                if rt == rt_count - 2:
                    # ship tiles 0..rt_count-2 while the last tile's matmuls
                    # run: only the last tile's two half-columns remain on
                    # the critical path, so the big DMA's HBM write receipt
                    # completes before the epilogue barrier needs it
                    nc.sync.dma_start(
                        out[:, :rt_count - 1], acc[:, :rt_count - 1])

            nc.sync.dma_start(out[:, rt_count - 1:], acc[:, rt_count - 1:])

    nc.compile()
    return nc


def _get_program(with_bias=False):
    key = ("bias" if with_bias else "fast")
    if key not in _CACHE:
        _CACHE[key] = _build_program(with_bias=with_bias)
    return _CACHE[key]


def _prep_e(emb):
    # [N, D] fp32 -> per-core ([NFIRST, p, kc, 128], [ng, p, kc, GRP*128])
    # fp16; contiguous per (group, partition) for 128-descriptor DMAs
    et = np.ascontiguousarray(
        np.asarray(emb, dtype=np.float32).T).astype(np.float16)
    # et [D, N]: [kc*128+p, c*R + rt*P + r]
    v = et.reshape(KC, P, NCORES, RT, P).transpose(2, 3, 1, 0, 4)
    # v [c, rt, p, kc, r]
    first = np.ascontiguousarray(v[:, :NFIRST])
    rest = np.ascontiguousarray(
        v[:, NFIRST:].reshape(NCORES, NG, GRP, P, KC, P)
        .transpose(0, 1, 3, 4, 2, 5)
        .reshape(NCORES, NG, P, KC, GRP * P))
    return first, rest


def _prep_w(W1, W2):
    # -> [p, kc, j, f] fp16: one 2KB-contiguous chunk per (partition, kc)
    w = np.stack([
        np.asarray(W1, dtype=np.float32).astype(np.float16),
        np.asarray(W2, dtype=np.float32).astype(np.float16),
    ])  # [j, kc*128+p, f]
    return np.ascontiguousarray(
        w.reshape(2, KC, P, F).transpose(2, 1, 0, 3))


def make_in_maps(emb_1, emb_2, W1, b1, W2, b2, with_bias=False):
    e1f, e1r = _prep_e(emb_1)
    e2f, e2r = _prep_e(emb_2)
    wb = _prep_w(W1, W2)
    maps = []
    for c in range(NCORES):
        m = {"e1f": e1f[c], "e2f": e2f[c], "e1h": e1r[c], "e2h": e2r[c],
             "wb": wb}
        if with_bias:
            m["b1"] = np.ascontiguousarray(np.asarray(b1, dtype=np.float32))
            m["b2"] = np.ascontiguousarray(np.asarray(b2, dtype=np.float32))
        maps.append(m)
    return maps


def kernel(emb_1, emb_2, W1, b1, W2, b2, **_unused):
    with_bias = bool(np.any(np.asarray(b1)) or np.any(np.asarray(b2)))
    nc = _get_program(with_bias)
    in_maps = make_in_maps(emb_1, emb_2, W1, b1, W2, b2, with_bias=with_bias)
    last_err = None
    for attempt in range(3):
        try:
            res = run_bass_kernel_spmd(nc, in_maps, list(range(NCORES))).results
            # out[p, rt] -> rows rt*128+p; fold in the last tile's second
            # half-reduction from the extra column
            outs = []
            for c in range(NCORES):
                raw = np.asarray(res[c]["out"])
                core = raw[:, :RT].copy()
                core[:, RT - 1] += raw[:, RT]
                outs.append(core.T.reshape(R))
            return np.concatenate(outs)
        except Exception as e:  # transient NRT/axon failures observed; retry
            last_err = e
            time.sleep(2.0 * (attempt + 1))
    raise last_err


# revision 42
# speedup vs baseline: 1.0233x; 1.0100x over previous
"""TRN2 Bass kernel for nn_BetweenClusterFC.

Computes out[n] = sum_f (emb_1 @ W1 + b1)[n,f] * (emb_2 @ W2 + b2)[n,f]
for emb_1/emb_2 [32768, 1024] fp32, W [1024, 512], b [512], out [32768] fp32.

Sharding: data-parallel over the 8 NeuronCores — each core handles 4096 rows;
W1/W2 replicated. No cross-core communication; outputs concatenated on the
host.

Strategy (single-pass fp16, DMA-descriptor-lean):
  - The rel-err budget for this problem is 2e-2; single-pass fp16 matmuls
    land at 3.4e-4 (measured on HW), so one fp16 matmul per (tile, kc,
    input) = 512 MMs of N=512 per core runs at the warm-PE stream roofline
    (216 ns/MM measured; 110.6us of stream per core).
  - Embeddings are relaid out host-side to [group, p, kc, r] so each e-DMA
    is 128 descriptors of >=2KB contiguous per partition (a plain [D, N]
    transpose gives 1024x512B descriptors, which cost 1.6us of sync-engine
    issue time each and stalled the PE). W1/W2 are interleaved host-side to
    [p, kc, j, f] and DMA'd in per-kc chunks in consumption order: startup
    is HBM-paced (~3MB must land before tile 0 completes, ~8.5us after the
    ~7us NEFF preamble), and the SDMA round-robins queued transfers, so
    fine chunks in consumption order minimize the PE wait.
  - Per 128-row tile: 16 j-interleaved matmuls accumulate h1/h2 into two
    PSUM banks (bufs=3 rotation so the PE never waits on the reducers);
    the scalar engine stages h1 PSUM->SBUF (the DVE may read only one PSUM
    operand), then DVE tensor_tensor + tensor_reduce write sum_f(h1*h2)
    into acc[:, rt]. (A fused scalar_tensor_tensor/tensor_tensor_reduce
    crashed the exec unit on HW, so the two-op form stays.) Biases are
    zero in this problem; a general bias variant compiles only if b != 0.
  - acc [128 rows-in-tile, 32 tiles] is DMA'd out raw and transposed on
    the host (free), eliminating the PE-transpose + copy + strided-store
    tail.
  - fp16 warmup matmuls on a zeroed tile (alternating two PSUM banks)
    bridge the startup-DMA window so real matmuls start at the
    un-throttled 2.4 GHz PE clock (HAM re-throttles after ~3.4us idle).
  - Measured: 133.7us HW exec (vs 368.6us baseline). Note the chip
    sometimes sits in a ~2.0 GHz power state (P0), where the same kernel
    measures ~158us; both states are ~stream-roofline-bound.
"""

import sys
import time

import numpy as np

if "/opt/trn_rl_repo" not in sys.path:
    sys.path.insert(0, "/opt/trn_rl_repo")

import concourse.mybir as mybir
import concourse.tile as tile
from concourse import bacc
from concourse.bass_utils import run_bass_kernel_spmd

F32 = mybir.dt.float32
F16 = mybir.dt.float16

N = 32768
D = 1024
F = 512
P = 128
NCORES = 8
R = N // NCORES   # rows per core
RT = R // P       # 128-row tiles per core
KC = D // P       # contraction chunks
NFIRST = 2        # single-tile e-DMA groups at the head (fast first arrival)
GRP = 2           # row-tiles per e-DMA group for the rest
NG = (RT - NFIRST) // GRP  # rest e-DMA groups
NWARM = 12        # fp16 warmup matmuls: spans the full startup-DMA window
                  # (ends ~11.9us, data ~12.8us) so the PE clock never
                  # re-throttles across the warmup -> first-matmul gap
DEPTH = 3         # A/B software-pipeline depth (== h_psum bufs; 6 banks)
FUSE_STT = False  # fused DVE mult+reduce crashed HW intermittently; keep off

_CACHE = {}


def _build_program(with_bias=False, rows=R):
    rt_count = rows // P
    ng = (rt_count - NFIRST) // GRP
    nc = bacc.Bacc("TRN2", target_bir_lowering=False, debug=False)

    def din(name, shape, dt=F16):
        return nc.dram_tensor(name, shape, dt, kind="ExternalInput").ap()

    # host-prearranged layouts: e [group, p, kc, r-in-group], w [p, kc, f]
    e1f = din("e1f", [NFIRST, P, KC, P])
    e2f = din("e2f", [NFIRST, P, KC, P])
    e1h = din("e1h", [ng, P, KC, GRP * P])
    e2h = din("e2h", [ng, P, KC, GRP * P])
    # W1/W2 interleaved per-kc so one DMA delivers both inputs' chunk k
    wb = din("wb", [P, KC, 2, F])
    if with_bias:
        b1 = din("b1", [F], F32)
        b2 = din("b2", [F], F32)
    # out[p, rt] = result for row rt*128+p; transposed host-side. One extra
    # column holds the last tile's second half-reduction (host adds it).
    out = nc.dram_tensor(
        "out", [P, rt_count + 1], F32, kind="ExternalOutput").ap()

    mult = mybir.AluOpType.mult
    add = mybir.AluOpType.add

    with tile.TileContext(nc) as tc:
        with (
            tc.tile_pool(name="consts", bufs=1) as consts,
            tc.tile_pool(name="epool", bufs=1) as epool,
            tc.tile_pool(name="hpool", bufs=2) as hpool,
            tc.tile_pool(name="fin", bufs=1) as fin_pool,
            tc.tile_pool(name="w_psum", bufs=1, space="PSUM") as w_psum,
            tc.tile_pool(name="h_psum", bufs=3, space="PSUM") as h_psum,
        ):
            wsb = consts.tile([P, KC, 2, F], F16, tag="wsb")
            ef = [
                [epool.tile([P, KC, P], F16, tag=f"e{j}f{t}", name=f"e{j}f{t}")
                 for t in range(NFIRST)]
                for j in range(2)
            ]
            eg = [
                [epool.tile([P, KC, GRP * P], F16, tag=f"e{j}g{g}",
                            name=f"e{j}g{g}")
                 for g in range(ng)]
                for j in range(2)
            ]

            # Startup is HBM-paced (~3MB before any tile can fully finish,
            # ~8.5us after the ~7us preamble) and the SDMA round-robins all
            # queued transfers, so delivery order ~= issue order at ~1us per
            # 256KB chunk. Issue in consumption order of the software-
            # pipelined A/B schedule below: first e tiles + kc0-3 weight
            # chunks, then the rest. Each DMA is 128 descriptors of >=2KB
            # contiguous per partition.
            # wc0/wc1 lead so the first real matmul is gated by the e tiles
            # (~11.5us), not a weight chunk queued behind them; e1f1/e2f1
            # slot in mid-weights so tile 1 never waits. (Splitting the
            # weight chunks across both HWDGE rings measured worse — the
            # rings share SDMA bandwidth, so it only reshuffles lateness.)
            nc.sync.dma_start(wsb[:, 0], wb[:, 0])
            nc.sync.dma_start(wsb[:, 1], wb[:, 1])
            nc.sync.dma_start(ef[0][0][:], e1f[0])
            nc.sync.dma_start(ef[1][0][:], e2f[0])
            nc.sync.dma_start(wsb[:, 2], wb[:, 2])
            nc.sync.dma_start(wsb[:, 3], wb[:, 3])
            for t in range(1, NFIRST):
                nc.sync.dma_start(ef[0][t][:], e1f[t])
                nc.sync.dma_start(ef[1][t][:], e2f[t])
            for k in range(4, KC):
                nc.sync.dma_start(wsb[:, k], wb[:, k])
            for g in range(ng):
                nc.sync.dma_start(eg[0][g][:], e1h[g])
                nc.sync.dma_start(eg[1][g][:], e2h[g])

            if with_bias:
                b1_bc = consts.tile([P, F], F32, tag="b1")
                nc.gpsimd.dma_start(b1_bc[:], b1[None, :].to_broadcast((P, F)))
                b2_bc = consts.tile([P, F], F32, tag="b2")
                nc.gpsimd.dma_start(b2_bc[:], b2[None, :].to_broadcast((P, F)))

            # fp16 warmup matmuls bridge the startup-DMA window so the first
            # real matmuls run at the un-throttled PE clock; alternate two
            # PSUM banks so consecutive warmups overlap fill/drain
            warm16 = consts.tile([P, F], F16, tag="warm16")
            nc.vector.memset(warm16[:], 0.0)
            warm_ps = [w_psum.tile([P, F], F32, tag=f"warm{i}", name=f"warm{i}")
                       for i in range(2)]
            for i in range(NWARM):
                nc.tensor.matmul(
                    warm_ps[i % 2][:], lhsT=warm16[:, :P], rhs=warm16[:],
                    start=True, stop=True,
                )

            acc = fin_pool.tile([P, rt_count + 1], F32, tag="acc")
            nc.gpsimd.memset(acc[:, rt_count:], 0.0)

            def lhs_of(rt, j, kc):
                if rt < NFIRST:
                    return ef[j][rt][:, kc, :]
                g, ri = divmod(rt - NFIRST, GRP)
                return eg[j][g][:, kc, ri * P:(ri + 1) * P]

            def mm_half(rt, hps, klo, khi, jmajor=False):
                # jmajor: finish j=0's accumulation first so the scalar-
                # engine h0 copy (and then the DVE chain) starts ~0.9us
                # before the half ends — the pipelined A-half that reuses
                # these PSUM banks then never waits on the DVE.
                order = ([(j, kc) for j in range(2)
                          for kc in range(klo, khi)] if jmajor else
                         [(j, kc) for kc in range(klo, khi)
                          for j in range(2)])
                for j, kc in order:
                    nc.tensor.matmul(
                        hps[j][:],
                        lhsT=lhs_of(rt, j, kc),
                        rhs=wsb[:, kc, j, :],
                        start=(kc == 0),
                        stop=(kc == KC - 1),
                    )

            def finish_tile_split(rt, hps):
                # last tile: pipeline the reduce in F-halves so the output
                # DMA issues ~0.7us sooner. The two half-reductions land in
                # acc[:, rt] and acc[:, rt+1]; the host adds them.
                for lo, hi, col in ((0, F // 2, rt), (F // 2, F, rt + 1)):
                    h0sb = hpool.tile([P, F // 2], F32, tag=f"h0s{lo}",
                                      name=f"h0s{lo}_{rt}")
                    nc.scalar.activation(
                        h0sb[:], hps[0][:, lo:hi],
                        mybir.ActivationFunctionType.Copy)
                    prod = hpool.tile([P, F // 2], F32, tag=f"pr{lo}",
                                      name=f"pr{lo}_{rt}")
                    nc.vector.tensor_tensor(
                        prod[:], h0sb[:], hps[1][:, lo:hi], mult)
                    nc.vector.tensor_reduce(
                        acc[:, col:col + 1], prod[:],
                        axis=mybir.AxisListType.X, op=add,
                    )

            def finish_tile(rt, hps):
                if with_bias:
                    hts = []
                    for j, b_bc in enumerate((b1_bc, b2_bc)):
                        ht = hpool.tile([P, F], F32, tag=f"ht{j}",
                                        name=f"ht{j}_{rt}")
                        nc.vector.tensor_tensor(ht[:], hps[j][:], b_bc[:], add)
                        hts.append(ht)
                    in0, in1 = hts[0][:], hts[1][:]
                else:
                    # DVE can read at most one PSUM operand; stage h0 in SBUF
                    # via the scalar engine (close to PSUM, off the DVE path)
                    h0sb = hpool.tile([P, F], F32, tag="h0sb",
                                      name=f"h0sb_{rt}")
                    nc.scalar.activation(
                        h0sb[:], hps[0][:], mybir.ActivationFunctionType.Copy)
                    in0, in1 = h0sb[:], hps[1][:]

                prod = hpool.tile([P, F], F32, tag="prod", name=f"prod_{rt}")
                if FUSE_STT:
                    nc.vector.scalar_tensor_tensor(
                        prod[:], in0, 1.0, in1, op0=mult, op1=mult,
                        accum_out=acc[:, rt:rt + 1],
                    )
                else:
                    nc.vector.tensor_tensor(prod[:], in0, in1, mult)
                    nc.vector.tensor_reduce(
                        acc[:, rt:rt + 1], prod[:],
                        axis=mybir.AxisListType.X, op=add,
                    )

            # Flat tile loop (measured faster than an A/B K-half software
            # pipeline, which traded startup idle for steady-state PSUM
            # reuse stalls): per tile, 16 j-interleaved matmuls accumulate
            # h1/h2 into two PSUM banks, then the scalar+vector engines
            # reduce into acc while the next tile's matmuls run.
            for rt in range(rt_count):
                hps = [
                    h_psum.tile([P, F], F32, tag=f"h{j}", name=f"hp{j}_{rt}")
                    for j in range(2)
                ]
                mm_half(rt, hps, 0, KC)
                if rt == rt_count - 1 and not with_bias:
                    finish_tile_split(rt, hps)
                else:
                    finish_tile(rt, hps)
                if rt == rt_count - 2:
                    # ship tiles 0..rt_count-2 while the last tile's matmuls
                    # run: only the last tile's two half-columns remain on
                    # the critical path, so the big DMA's HBM write receipt
                    # completes before the epilogue barrier needs it
                    nc.sync.dma_start(
                        out[:, :rt_count - 1], acc[:, :rt_count - 1])

            nc.sync.dma_start(out[:, rt_count - 1:], acc[:, rt_count - 1:])

    nc.compile()
    return nc


def _get_program(with_bias=False):
    key = ("bias" if with_bias else "fast")
    if key not in _CACHE:
        _CACHE[key] = _build_program(with_bias=with_bias)
    return _CACHE[key]


def _prep_e(emb):
    # [N, D] fp32 -> per-core ([NFIRST, p, kc, 128], [ng, p, kc, GRP*128])
    # fp16; contiguous per (group, partition) for 128-descriptor DMAs
    et = np.ascontiguousarray(
        np.asarray(emb, dtype=np.float32).T).astype(np.float16)
    # et [D, N]: [kc*128+p, c*R + rt*P + r]
    v = et.reshape(KC, P, NCORES, RT, P).transpose(2, 3, 1, 0, 4)
    # v [c, rt, p, kc, r]
    first = np.ascontiguousarray(v[:, :NFIRST])
    rest = np.ascontiguousarray(
        v[:, NFIRST:].reshape(NCORES, NG, GRP, P, KC, P)
        .transpose(0, 1, 3, 4, 2, 5)
        .reshape(NCORES, NG, P, KC, GRP * P))
    return first, rest


def _prep_w(W1, W2):
    # -> [p, kc, j, f] fp16: one 2KB-contiguous chunk per (partition, kc)
    w = np.stack([
        np.asarray(W1, dtype=np.float32).astype(np.float16),
        np.asarray(W2, dtype=np.float32).astype(np.float16),
    ])  # [j, kc*128+p, f]
    return np.ascontiguousarray(
        w.reshape(2, KC, P, F).transpose(2, 1, 0, 3))


def make_in_maps(emb_1, emb_2, W1, b1, W2, b2, with_bias=False):
    e1f, e1r = _prep_e(emb_1)
    e2f, e2r = _prep_e(emb_2)
    wb = _prep_w(W1, W2)
    maps = []
    for c in range(NCORES):
        m = {"e1f": e1f[c], "e2f": e2f[c], "e1h": e1r[c], "e2h": e2r[c],
             "wb": wb}
        if with_bias:
            m["b1"] = np.ascontiguousarray(np.asarray(b1, dtype=np.float32))
            m["b2"] = np.ascontiguousarray(np.asarray(b2, dtype=np.float32))
        maps.append(m)
    return maps


def kernel(emb_1, emb_2, W1, b1, W2, b2, **_unused):
    with_bias = bool(np.any(np.asarray(b1)) or np.any(np.asarray(b2)))
    nc = _get_program(with_bias)
    in_maps = make_in_maps(emb_1, emb_2, W1, b1, W2, b2, with_bias=with_bias)
    last_err = None
    for attempt in range(3):
        try:
            res = run_bass_kernel_spmd(nc, in_maps, list(range(NCORES))).results
            # out[p, rt] -> rows rt*128+p; fold in the last tile's second
            # half-reduction from the extra column
            outs = []
            for c in range(NCORES):
                raw = np.asarray(res[c]["out"])
                core = raw[:, :RT].copy()
                core[:, RT - 1] += raw[:, RT]
                outs.append(core.T.reshape(R))
            return np.concatenate(outs)
        except Exception as e:  # transient NRT/axon failures observed; retry
            last_err = e
            time.sleep(2.0 * (attempt + 1))
    raise last_err
